# revision 32
# baseline (speedup 1.0000x reference)
"""AttentionBlock (GroupNorm + 8-head self-attention + proj + residual) on 8 trn2 cores.

Sharding: data-parallel over batch (B=8 -> 1 sample per core). No collectives.

v2 design (per core, one sample; C=512, N=HW=1024, 8 heads, hd=64):
  Precision plan (validated end-to-end on host, rel err ~9e-3 < 2e-2):
    x arrives twice: bf16 (fast startup, feeds GroupNorm) + f32 (lazy, residual).
    qkv + proj matmuls run in fp8e4m3 with DoubleRow (K=256/instr, 2x rate).
    scores/AV run bf16; exp splits across two engines:
      ACT: true exp (table), head-A tiles; DVE: Schraudolph bits-trick exp
      (one tensor_scalar: round(s*128*log2e/8 + 128*(127-.043)) as int16,
      bitcast to bf16; verified bit-exact vs round() model on HW).
  PE: scores row-tiled 2x via tile_position (0,0)/(64,0) -- the two heads of a
    pair contract K=64 on separate array halves concurrently. AV keeps the
    vt-stationary [128, 65] form (ones column = softmax denominator for free).
  Engine split: ACT = exp-A + q/k PSUM evac (Copy+bias); DVE = exp-B +
    GN apply + reciprocals + residual; GPSIMD = denominator broadcast +
    attn normalize multiply; PE fed by a flat software-pipelined stream.
"""

import sys

sys.path.insert(0, "/opt/trn_rl_repo")

import contextlib

import ml_dtypes
import numpy as np

import concourse.bass as bass
import concourse.tile as tile
from concourse import bacc, mybir
from concourse.bass_utils import run_bass_kernel_spmd

f32 = mybir.dt.float32
bf16 = mybir.dt.bfloat16
i16 = mybir.dt.int16
fp8 = mybir.dt.float8e4
AF = mybir.ActivationFunctionType
OP = mybir.AluOpType
PM = mybir.MatmulPerfMode

C = 512
N = 1024
NHEADS = 8
HD = 64
GROUPS = 32
GSIZE = 16  # channels per group
CT = 4  # c-tiles of 128
MT = 8  # m-tiles of 128
PAIRS = 4  # head pairs
EPS = 1e-5
NCHUNK = 512
P = 128

LOG2E = 1.4426950408889634
SCH_S = 128.0 * LOG2E / 8.0          # fold the 1/sqrt(hd)=1/8 score scale
SCH_B = 128.0 * (127.0 - 0.043)


def build_program():
    nc = bacc.Bacc("TRN2", target_bir_lowering=False, debug=True)

    xb_d = nc.dram_tensor("xb", [C, N], bf16, kind="ExternalInput")
    xf_d = nc.dram_tensor("xf", [C, N], f32, kind="ExternalInput")
    w8_d = nc.dram_tensor("w8", [2, P, 2, 3 * C], fp8, kind="ExternalInput")
    wp8_d = nc.dram_tensor("wp8", [2, P, 2, C], fp8, kind="ExternalInput")
    # packed [gnw(4) | gnb(4) | gmap(8) | qkb(8) | pb(4)] -- one DMA
    cpack_d = nc.dram_tensor("cpack", [P, 28], f32, kind="ExternalInput")
    gmapT_d = nc.dram_tensor("gmapT", [8, P], f32, kind="ExternalInput")
    vb_d = nc.dram_tensor("vb", [1, C], bf16, kind="ExternalInput")
    out_d = nc.dram_tensor("out", [C, N], f32, kind="ExternalOutput")

    with tile.TileContext(nc) as tc, contextlib.ExitStack() as ctx:
        consts = ctx.enter_context(tc.tile_pool(name="consts", bufs=1))
        xbp = ctx.enter_context(tc.tile_pool(name="xbp", bufs=CT))
        xfp = ctx.enter_context(tc.tile_pool(name="xfp", bufs=CT))
        xn8p = ctx.enter_context(tc.tile_pool(name="xn8p", bufs=2))
        w8p = ctx.enter_context(tc.tile_pool(name="w8p", bufs=2))
        wp8p = ctx.enter_context(tc.tile_pool(name="wp8p", bufs=2))
        qkp = ctx.enter_context(tc.tile_pool(name="qkp", bufs=4))
        vtp = ctx.enter_context(tc.tile_pool(name="vtp", bufs=MT))
        att8p = ctx.enter_context(tc.tile_pool(name="att8p", bufs=2))
        expp = ctx.enter_context(tc.tile_pool(name="expp", bufs=14))
        dvp = ctx.enter_context(tc.tile_pool(name="dvp", bufs=4))
        gnp = ctx.enter_context(tc.tile_pool(name="gnp", bufs=4))
        outp = ctx.enter_context(tc.tile_pool(name="outp", bufs=2))

        psum_big = ctx.enter_context(tc.tile_pool(name="psum_big", bufs=2, space="PSUM"))

        # ---- input DMAs: tiny consts FIRST and batched (GroupNorm needs
        # gmap/gnw immediately; per-DMA issue cost is ~600ns on the queue) ----
        cpack_t = consts.tile([P, 28], f32)
        nc.sync.dma_start(cpack_t[:], cpack_d[:])
        gmapT_t = consts.tile([8, P], f32)
        nc.sync.dma_start(gmapT_t[:], gmapT_d[:])
        ones8_t = consts.tile([P, 8], bf16)
        nc.vector.memset(ones8_t[:], 1.0)
        ones1_t = consts.tile([1, P], bf16)
        nc.vector.memset(ones1_t[:], 1.0)

        xb_tiles = []
        for t in range(CT):
            xt = xbp.tile([P, N], bf16, tag="xb")
            nc.sync.dma_start(xt[:], xb_d[t * P:(t + 1) * P, :])
            xb_tiles.append(xt)

        w8_tiles = []
        for s in range(2):
            wt = w8p.tile([P, 2, 3 * C], fp8, tag="w8")
            nc.sync.dma_start(wt[:], w8_d[s])
            w8_tiles.append(wt)
        eps_t = consts.tile([8, 1], f32)
        nc.vector.memset(eps_t[:], EPS)
        # preload the rsqrt ACT table set early (GroupNorm needs it first)
        warm_t = consts.tile([1, 1], f32)
        nc.vector.memset(warm_t[:], 1.0)
        nc.scalar.activation(out=warm_t[:], in_=warm_t[:], func=AF.Sqrt)
        zd_t = consts.tile([P, NCHUNK], bf16)
        nc.vector.memset(zd_t[:], 0.0)

        # second DMA ring: v bias, proj weights + f32 x (residual; needed late)
        vb_t = consts.tile([1, C], bf16)
        nc.scalar.dma_start(vb_t[:], vb_d[:])
        wp8_tiles = []
        for s in range(2):
            wt = wp8p.tile([P, 2, C], fp8, tag="wp8")
            nc.scalar.dma_start(wt[:], wp8_d[s])
            wp8_tiles.append(wt)
        xf_tiles = []
        for t in range(CT):
            xt = xfp.tile([P, N], f32, tag="xf")
            for hh in range(2):
                nc.scalar.dma_start(
                    xt[:, hh * NCHUNK:(hh + 1) * NCHUNK],
                    xf_d[t * P:(t + 1) * P, hh * NCHUNK:(hh + 1) * NCHUNK],
                )
            xf_tiles.append(xt)

        def pe_warm(n):
            for _ in range(n):
                dp = psum_big.tile([P, N], f32, tag="big", name="dummy")
                nc.tensor.matmul(
                    dp[:, 0:NCHUNK], zd_t[:, 0:P], zd_t[:], start=True, stop=True
                )

        # ---- GroupNorm -> xn8 fp8 mega-tiles [128, 2, 1024] ----
        xn8_tiles = [
            xn8p.tile([P, 2, N], fp8, tag="xn8", name=f"xn8_{s}") for s in range(2)
        ]
        with tc.tile_pool(name="psum_tiny", bufs=1, space="PSUM") as psum_tiny:
            pe_warm(3)
            for t in range(CT):
                xt = xb_tiles[t]
                xv = xt[:].rearrange("p (s f) -> p s f", s=2)
                st = gnp.tile([P, 2, 6], f32, tag="bnst")
                nc.vector.bn_stats(out=st[:, 0, :], in_=xv[:, 0, :])
                nc.vector.bn_stats(out=st[:, 1, :], in_=xv[:, 1, :])
                mv = gnp.tile([P, 2], f32, tag="bnmv")
                nc.vector.bn_aggr(out=mv[:], in_=st[:])
                cst = gnp.tile([P, 2], f32, tag="cst")
                nc.vector.tensor_copy(cst[:, 0:1], mv[:, 0:1])
                nc.vector.scalar_tensor_tensor(
                    out=cst[:, 1:2], in0=mv[:, 0:1], scalar=mv[:, 0:1],
                    in1=mv[:, 1:2], op0=OP.mult, op1=OP.add,
                )
                pgs = psum_tiny.tile([8, 2], f32, tag="pgs")
                nc.tensor.matmul(pgs[:], cpack_t[:, 8:16], cst[:], start=True, stop=True)
                gr = gnp.tile([8, 2], f32, tag="gr")
                nc.vector.tensor_scalar_mul(gr[:, 0:2], pgs[:, 0:2], 1.0 / GSIZE)
                musq = gnp.tile([8, 1], f32, tag="musq")
                nc.vector.tensor_mul(musq[:], gr[:, 0:1], gr[:, 0:1])
                var = gnp.tile([8, 1], f32, tag="var")
                nc.vector.tensor_sub(var[:], gr[:, 1:2], musq[:])
                std = gnp.tile([8, 1], f32, tag="std")
                nc.scalar.activation(
                    out=std[:], in_=var[:], func=AF.Sqrt, bias=eps_t[:]
                )
                nc.vector.reciprocal(gr[:, 1:2], std[:])
                pbc = psum_tiny.tile([P, 2], f32, tag="pbc")
                nc.tensor.matmul(pbc[:], gmapT_t[:], gr[:], start=True, stop=True)
                scale_c = gnp.tile([P, 1], f32, tag="scale_c")
                nc.vector.tensor_mul(scale_c[:], pbc[:, 1:2], cpack_t[:, t:t + 1])
                mss = gnp.tile([P, 1], f32, tag="mss")
                nc.vector.tensor_mul(mss[:], pbc[:, 0:1], scale_c[:])
                bias_c = gnp.tile([P, 1], f32, tag="bias_c")
                nc.vector.tensor_sub(bias_c[:], cpack_t[:, CT + t:CT + t + 1], mss[:])
                nc.vector.tensor_scalar(
                    out=xn8_tiles[t // 2][:, t % 2, :], in0=xt[:],
                    scalar1=scale_c[:], scalar2=bias_c[:],
                    op0=OP.mult, op1=OP.add,
                )
                pe_warm(2)
            # switch the ACT table set to exp now (overlaps the qkv phase).
            # Input depends on the last GN sqrt so the scheduler cannot hoist
            # this above the sqrts (which would thrash table loads).
            nc.scalar.activation(out=warm_t[:], in_=std[0:1, :], func=AF.Exp)

        with tc.tile_pool(name="psum_av", bufs=2, space="PSUM") as psum_av:

            # ---- qkv helpers (fp8 DoubleRow) ----
            def emit_vt_tile(i):
                """vT tile [128, 8, 65]: per head h cols 0:64 = v chans, col 64
                = ones (softmax denominator via the AV matmul)."""
                ps = psum_big.tile([P, N], f32, tag="big", name=f"vtps{i}")
                pv = ps[:, 0:NCHUNK]
                for s in range(2):
                    nc.tensor.matmul(
                        pv,
                        xn8_tiles[s][:, :, i * P:(i + 1) * P],
                        w8_tiles[s][:, :, 2 * C:3 * C],
                        start=(s == 0), stop=False,
                        perf_mode=PM.DoubleRow,
                    )
                nc.tensor.matmul(pv, ones1_t[:], vb_t[:], start=False, stop=True)
                vt = vtp.tile([P, NHEADS, HD + 1], bf16, tag="vt", name=f"vt{i}")
                nc.scalar.activation(
                    out=vt[:, :, 0:HD], in_=pv.rearrange("p (h d) -> p h d", h=NHEADS),
                    func=AF.Identity,
                )
                nc.vector.tensor_copy(vt[:, :, HD:HD + 1], ones8_t[:, :, None])
                return vt

            # qk generation: per pair, q tile then k tile (fp8 DR, evac on ACT)
            qk_state = {}

            def qk_begin(p):
                qk_state[p] = {"chunk": 0, "ps": None, "sb": []}

            def qk_chunk(p):
                """Emit one DR matmul (of 8) for pair p; 4 per psum (s x j),
                q fully first then k. After each psum completes, ACT evacuates
                with the per-partition bias."""
                st = qk_state[p]
                c = st["chunk"]
                if c >= 8:
                    return
                st["chunk"] = c + 1
                which, cc = c // 4, c % 4
                off = which * C + p * P
                if cc == 0:
                    st["ps"] = psum_big.tile(
                        [P, N], f32, tag="big", name=f"qkps{p}_{which}"
                    )
                ps = st["ps"]
                s, j = cc // 2, cc % 2
                nc.tensor.matmul(
                    ps[:, j * NCHUNK:(j + 1) * NCHUNK],
                    w8_tiles[s][:, :, off:off + P],
                    xn8_tiles[s][:, :, j * NCHUNK:(j + 1) * NCHUNK],
                    start=(s == 0), stop=(s == 1),
                    perf_mode=PM.DoubleRow,
                )
                if cc == 3:
                    sb = qkp.tile([P, N], bf16, tag="qk", name=f"qk{p}_{which}")
                    nc.scalar.activation(
                        out=sb[:], in_=ps[:], func=AF.Identity,
                        bias=cpack_t[:, 16 + which * 4 + p:17 + which * 4 + p],
                    )
                    st["sb"].append(sb)

            def qk_force(p):
                while qk_state[p]["chunk"] < 8:
                    qk_chunk(p)

            # ---- scores + exp (row-tiled PE pairs; ACT/DVE split) ----
            # DVE handles head B's exp via the Schraudolph bit trick, except
            # the m-steps listed in ACT_B (rebalance knob).
            ACT_B = (2, 5)
            exps = {}
            emitted = 0
            steps = [(p, i) for p in range(PAIRS) for i in range(MT)]

            def emit_scores_exp(p, i, q_t, k_t):
                psA = psum_big.tile([P, N], f32, tag="big", name=f"sA{p}_{i}")
                psB = psum_big.tile([P, N], f32, tag="big", name=f"sB{p}_{i}")
                for j in range(2):
                    nc.tensor.matmul(
                        psA[:, j * NCHUNK:(j + 1) * NCHUNK],
                        k_t[0:HD, i * P:(i + 1) * P],
                        q_t[0:HD, j * NCHUNK:(j + 1) * NCHUNK],
                        start=True, stop=True, tile_position=(0, 0),
                    )
                    nc.tensor.matmul(
                        psB[:, j * NCHUNK:(j + 1) * NCHUNK],
                        k_t[HD:P, i * P:(i + 1) * P],
                        q_t[HD:P, j * NCHUNK:(j + 1) * NCHUNK],
                        start=True, stop=True, tile_position=(64, 0),
                    )
                eA = expp.tile([P, N], bf16, tag="exp", name=f"eA{p}_{i}")
                nc.scalar.activation(out=eA[:], in_=psA[:], func=AF.Exp, scale=1.0 / 8.0)
                eB = expp.tile([P, N], bf16, tag="exp", name=f"eB{p}_{i}")
                if i in ACT_B:
                    nc.scalar.activation(out=eB[:], in_=psB[:], func=AF.Exp, scale=1.0 / 8.0)
                else:
                    nc.vector.tensor_scalar(
                        out=eB[:].bitcast(i16), in0=psB[:],
                        scalar1=SCH_S, scalar2=SCH_B, op0=OP.mult, op1=OP.add,
                    )
                return eA, eB

            def ensure_scores(n):
                nonlocal emitted
                while emitted < min(n, len(steps)):
                    p2, i2 = steps[emitted]
                    qk_force(p2)
                    exps[steps[emitted]] = emit_scores_exp(
                        p2, i2, *qk_state[p2]["sb"]
                    )
                    emitted += 1

            def emit_av(avt, p, i, h, start, stop):
                pair = exps[(p, i)]
                e = pair[h]
                if h == 1:
                    del exps[(p, i)]
                for j in range(2):
                    nc.tensor.matmul(
                        avt[:, j * NCHUNK:(j + 1) * NCHUNK],
                        vt_tiles[i][:, 2 * p + h, :],
                        e[:, j * NCHUNK:(j + 1) * NCHUNK],
                        start=start, stop=stop,
                    )

            def emit_norm(avt, p, h):
                """att8_mega[h'//4][(h'%2)*64 partitions, (h'//2)%2, :] =
                avt[0:64] * recip(avt[64]) for global head h' = 2p+h."""
                hh = 2 * p + h
                dinv = dvp.tile([1, N], f32, tag="dinv", name=f"dinv{hh}")
                nc.scalar.activation(out=dinv[:], in_=avt[HD:HD + 1, :], func=AF.Identity)
                nc.vector.reciprocal_approx_fast(dinv[:], dinv[:])
                dinvb = dvp.tile([HD, N], f32, tag="dinvb", name=f"dinvb{hh}")
                nc.gpsimd.partition_broadcast(dinvb[:], dinv[:])
                lo = (hh % 2) * HD
                nc.vector.tensor_mul(
                    att8_tiles[hh // 4][lo:lo + HD, (hh // 2) % 2, :],
                    avt[0:HD, :], dinvb[:],
                )

            att8_tiles = [
                att8p.tile([P, 2, N], fp8, tag="att8", name=f"att8_{m}")
                for m in range(2)
            ]

            # ---- flat software-pipelined stream ----
            LA = 1
            vt_tiles = [None] * MT
            proj_ps = {}
            qk_begin(0)
            qk_force(0)
            ensure_scores(LA)
            for i in range(MT):
                vt_tiles[i] = emit_vt_tile(i)
                if i in (2, 4, 6):
                    ensure_scores(LA + 1 + i // 2)

            def emit_proj(o, s, start, stop):
                if s == 0:
                    proj_ps[o] = psum_big.tile([P, N], f32, tag="big", name=f"pps{o}")
                pp = proj_ps[o]
                for j in range(2):
                    nc.tensor.matmul(
                        pp[:, j * NCHUNK:(j + 1) * NCHUNK],
                        wp8_tiles[s][:, :, o * P:(o + 1) * P],
                        att8_tiles[s][:, :, j * NCHUNK:(j + 1) * NCHUNK],
                        start=start, stop=stop,
                        perf_mode=PM.DoubleRow,
                    )

            for p in range(PAIRS):
                # head A AV trails the exp stream
                avt = psum_av.tile([HD + 1, N], f32, tag="av", name=f"avA{p}")
                for i in range(MT):
                    # last 2 steps: hold back exp lookahead so norm-A's ACT/DVE
                    # ops aren't queued behind lookahead exps (they gate the
                    # next pair's AV via the psum_av buffer rotation)
                    la = LA if i < MT - 2 else 0
                    ensure_scores(p * MT + i + 1 + la)
                    if p + 1 < PAIRS:
                        if i == 0:
                            qk_begin(p + 1)
                        qk_chunk(p + 1)
                    emit_av(avt, p, i, 0, start=(i == 0), stop=(i == MT - 1))
                emit_norm(avt, p, 0)
                ensure_scores(p * MT + MT + LA)
                if p == PAIRS - 1:
                    # all scores are emitted by now; big-pool slots are free.
                    # att8 mega 0 (heads 0-3) has been ready since pair 1 --
                    # pre-accumulate proj s=0 for 2 o-tiles as PE filler while
                    # the tail normalize chains run.
                    for o in range(2):
                        emit_proj(o, 0, start=True, stop=False)
                # head B AV blasts through retained exp tiles
                avt = psum_av.tile([HD + 1, N], f32, tag="av", name=f"avB{p}")
                for i in range(MT):
                    emit_av(avt, p, i, 1, start=(i == 0), stop=(i == MT - 1))
                    if i % 3 == 2:
                        ensure_scores(p * MT + MT + i // 3 + 1 + LA)
                emit_norm(avt, p, 1)

            # ---- proj s=1 + bias + residual + out DMA ----
            for o in range(CT):
                if o not in proj_ps:
                    emit_proj(o, 0, start=True, stop=False)
                emit_proj(o, 1, start=False, stop=True)
                ot = outp.tile([P, N], f32, tag="ot")
                nc.vector.scalar_tensor_tensor(
                    out=ot[:], in0=proj_ps[o][:], scalar=cpack_t[:, 24 + o:25 + o],
                    in1=xf_tiles[o][:], op0=OP.add, op1=OP.add,
                )
                nc.sync.dma_start(out_d[o * P:(o + 1) * P, :], ot[:])

    nc.compile()
    return nc


_CACHE = {}


def _get_program():
    if "nc" not in _CACHE:
        _CACHE["nc"] = build_program()
    return _CACHE["nc"]


def make_in_maps(x, gn_w, gn_b, qkv_w, qkv_b, proj_w, proj_b):
    B = x.shape[0]
    f = np.float32
    f8 = ml_dtypes.float8_e4m3
    # DoubleRow packing: contraction channel c -> (s=c//256, p=c%128, sub=(c//128)%2)
    wqkvT = np.ascontiguousarray(np.asarray(qkv_w, f).T)  # [512, 1536]
    w8 = np.ascontiguousarray(
        wqkvT.reshape(2, 2, P, 3 * C).transpose(0, 2, 1, 3)
    ).astype(f8)  # [2, 128, 2, 1536]
    wpT = np.ascontiguousarray(np.asarray(proj_w, f).T)  # [512, 512]
    wp8 = np.ascontiguousarray(
        wpT.reshape(2, 2, P, C).transpose(0, 2, 1, 3)
    ).astype(f8)  # [2, 128, 2, 512]
    qkb = np.asarray(qkv_b[:2 * C], f).reshape(8, P).T
    vb = np.asarray(qkv_b[2 * C:], f).reshape(1, C).astype(ml_dtypes.bfloat16)
    pb = np.asarray(proj_b, f).reshape(CT, P).T
    gnw = np.asarray(gn_w, f).reshape(CT, P).T
    gnb = np.asarray(gn_b, f).reshape(CT, P).T
    gmap = np.zeros((P, 8), f)
    gmap[np.arange(P), np.arange(P) // GSIZE] = 1.0
    gmapT = np.ascontiguousarray(gmap.T)
    cpack = np.ascontiguousarray(
        np.concatenate([gnw, gnb, gmap, qkb, pb], axis=1)
    )  # [128, 28]
    shared = dict(w8=w8, wp8=wp8, cpack=cpack, gmapT=gmapT, vb=vb)
    xs = np.asarray(x, f).reshape(B, C, N)
    return [
        dict(
            shared,
            xb=np.ascontiguousarray(xs[i]).astype(ml_dtypes.bfloat16),
            xf=np.ascontiguousarray(xs[i]),
        )
        for i in range(B)
    ]


def run(in_maps, trace=False, **kw):
    nc = _get_program()
    return run_bass_kernel_spmd(nc, in_maps, core_ids=list(range(len(in_maps))), trace=trace, **kw)


def kernel(x, gn_w, gn_b, qkv_w, qkv_b, proj_w, proj_b):
    x = np.asarray(x)
    B, c, h, w = x.shape
    in_maps = make_in_maps(x, gn_w, gn_b, qkv_w, qkv_b, proj_w, proj_b)
    res = run(in_maps)
    out = np.stack([res.results[i]["out"].reshape(c, h, w) for i in range(B)])
    return out.astype(np.float32)


# revision 35
# speedup vs baseline: 1.0248x; 1.0248x over previous
"""AttentionBlock (GroupNorm + 8-head self-attention + proj + residual) on 8 trn2 cores.

Sharding: data-parallel over batch (B=8 -> 1 sample per core). No collectives.

v2 design (per core, one sample; C=512, N=HW=1024, 8 heads, hd=64):
  Precision plan (validated end-to-end on host, rel err ~9e-3 < 2e-2):
    x arrives twice: bf16 (fast startup, feeds GroupNorm) + f32 (lazy, residual).
    qkv + proj matmuls run in fp8e4m3 with DoubleRow (K=256/instr, 2x rate).
    scores/AV run bf16; exp splits across two engines:
      ACT: true exp (table), head-A tiles; DVE: Schraudolph bits-trick exp
      (one tensor_scalar: round(s*128*log2e/8 + 128*(127-.043)) as int16,
      bitcast to bf16; verified bit-exact vs round() model on HW).
  PE: scores row-tiled 2x via tile_position (0,0)/(64,0) -- the two heads of a
    pair contract K=64 on separate array halves concurrently. AV keeps the
    vt-stationary [128, 65] form (ones column = softmax denominator for free).
  Engine split: ACT = exp-A + q/k PSUM evac (Copy+bias); DVE = exp-B +
    GN apply + reciprocals + residual; GPSIMD = denominator broadcast +
    attn normalize multiply; PE fed by a flat software-pipelined stream.
"""

import sys

sys.path.insert(0, "/opt/trn_rl_repo")

import contextlib

import ml_dtypes
import numpy as np

import concourse.bass as bass
import concourse.tile as tile
from concourse import bacc, mybir
from concourse.bass_utils import run_bass_kernel_spmd

f32 = mybir.dt.float32
bf16 = mybir.dt.bfloat16
i16 = mybir.dt.int16
fp8 = mybir.dt.float8e4
AF = mybir.ActivationFunctionType
OP = mybir.AluOpType
PM = mybir.MatmulPerfMode

C = 512
N = 1024
NHEADS = 8
HD = 64
GROUPS = 32
GSIZE = 16  # channels per group
CT = 4  # c-tiles of 128
MT = 8  # m-tiles of 128
PAIRS = 4  # head pairs
EPS = 1e-5
NCHUNK = 512
P = 128

LOG2E = 1.4426950408889634
SCH_S = 128.0 * LOG2E / 8.0          # fold the 1/sqrt(hd)=1/8 score scale
SCH_B = 128.0 * (127.0 - 0.043)


def build_program():
    nc = bacc.Bacc("TRN2", target_bir_lowering=False, debug=True)

    xb_d = nc.dram_tensor("xb", [C, N], bf16, kind="ExternalInput")
    xf_d = nc.dram_tensor("xf", [C, N], f32, kind="ExternalInput")
    w8_d = nc.dram_tensor("w8", [2, P, 2, 3 * C], fp8, kind="ExternalInput")
    wp8_d = nc.dram_tensor("wp8", [2, P, 2, C], fp8, kind="ExternalInput")
    # packed [gnw(4) | gnb(4) | gmap(8) | qkb(8) | pb(4)] -- one DMA
    cpack_d = nc.dram_tensor("cpack", [P, 28], f32, kind="ExternalInput")
    gmapT_d = nc.dram_tensor("gmapT", [8, P], f32, kind="ExternalInput")
    vb_d = nc.dram_tensor("vb", [1, C], bf16, kind="ExternalInput")
    out_d = nc.dram_tensor("out", [C, N], f32, kind="ExternalOutput")

    with tile.TileContext(nc) as tc, contextlib.ExitStack() as ctx:
        consts = ctx.enter_context(tc.tile_pool(name="consts", bufs=1))
        xbp = ctx.enter_context(tc.tile_pool(name="xbp", bufs=CT))
        xfp = ctx.enter_context(tc.tile_pool(name="xfp", bufs=CT))
        xn8p = ctx.enter_context(tc.tile_pool(name="xn8p", bufs=2))
        w8p = ctx.enter_context(tc.tile_pool(name="w8p", bufs=2))
        wp8p = ctx.enter_context(tc.tile_pool(name="wp8p", bufs=2))
        qkp = ctx.enter_context(tc.tile_pool(name="qkp", bufs=4))
        vtp = ctx.enter_context(tc.tile_pool(name="vtp", bufs=MT))
        att8p = ctx.enter_context(tc.tile_pool(name="att8p", bufs=2))
        expp = ctx.enter_context(tc.tile_pool(name="expp", bufs=14))
        dvp = ctx.enter_context(tc.tile_pool(name="dvp", bufs=4))
        gnp = ctx.enter_context(tc.tile_pool(name="gnp", bufs=4))
        outp = ctx.enter_context(tc.tile_pool(name="outp", bufs=2))

        psum_big = ctx.enter_context(tc.tile_pool(name="psum_big", bufs=2, space="PSUM"))

        # ---- input DMAs: tiny consts FIRST and batched (GroupNorm needs
        # gmap/gnw immediately; per-DMA issue cost is ~600ns on the queue) ----
        cpack_t = consts.tile([P, 28], f32)
        nc.sync.dma_start(cpack_t[:], cpack_d[:])
        gmapT_t = consts.tile([8, P], f32)
        nc.sync.dma_start(gmapT_t[:], gmapT_d[:])
        ones8_t = consts.tile([P, 8], bf16)
        nc.vector.memset(ones8_t[:], 1.0)
        ones1_t = consts.tile([1, P], bf16)
        nc.vector.memset(ones1_t[:], 1.0)

        xb_tiles = []
        for t in range(CT):
            xt = xbp.tile([P, N], bf16, tag="xb")
            nc.sync.dma_start(xt[:], xb_d[t * P:(t + 1) * P, :])
            xb_tiles.append(xt)

        w8_tiles = []
        for s in range(2):
            wt = w8p.tile([P, 2, 3 * C], fp8, tag="w8")
            nc.sync.dma_start(wt[:], w8_d[s])
            w8_tiles.append(wt)
        eps_t = consts.tile([8, 1], f32)
        nc.vector.memset(eps_t[:], EPS)
        # preload the rsqrt ACT table set early (GroupNorm needs it first)
        warm_t = consts.tile([1, 1], f32)
        nc.vector.memset(warm_t[:], 1.0)
        nc.scalar.activation(out=warm_t[:], in_=warm_t[:], func=AF.Sqrt)
        zd_t = consts.tile([P, NCHUNK], bf16)
        nc.vector.memset(zd_t[:], 0.0)

        # second DMA ring: v bias, proj weights + f32 x (residual; needed late)
        vb_t = consts.tile([1, C], bf16)
        nc.scalar.dma_start(vb_t[:], vb_d[:])
        wp8_tiles = []
        for s in range(2):
            wt = wp8p.tile([P, 2, C], fp8, tag="wp8")
            nc.scalar.dma_start(wt[:], wp8_d[s])
            wp8_tiles.append(wt)
        xf_tiles = []
        for t in range(CT):
            xt = xfp.tile([P, N], f32, tag="xf")
            for hh in range(2):
                nc.scalar.dma_start(
                    xt[:, hh * NCHUNK:(hh + 1) * NCHUNK],
                    xf_d[t * P:(t + 1) * P, hh * NCHUNK:(hh + 1) * NCHUNK],
                )
            xf_tiles.append(xt)

        def pe_warm(n):
            for _ in range(n):
                dp = psum_big.tile([P, N], f32, tag="big", name="dummy")
                nc.tensor.matmul(
                    dp[:, 0:NCHUNK], zd_t[:, 0:P], zd_t[:], start=True, stop=True
                )

        # ---- GroupNorm -> xn8 fp8 mega-tiles [128, 2, 1024] ----
        xn8_tiles = [
            xn8p.tile([P, 2, N], fp8, tag="xn8", name=f"xn8_{s}") for s in range(2)
        ]
        with tc.tile_pool(name="psum_tiny", bufs=1, space="PSUM") as psum_tiny:
            pe_warm(10)
            for t in range(CT):
                xt = xb_tiles[t]
                xv = xt[:].rearrange("p (s f) -> p s f", s=2)
                st = gnp.tile([P, 2, 6], f32, tag="bnst")
                nc.vector.bn_stats(out=st[:, 0, :], in_=xv[:, 0, :])
                nc.vector.bn_stats(out=st[:, 1, :], in_=xv[:, 1, :])
                mv = gnp.tile([P, 2], f32, tag="bnmv")
                nc.vector.bn_aggr(out=mv[:], in_=st[:])
                cst = gnp.tile([P, 2], f32, tag="cst")
                nc.vector.tensor_copy(cst[:, 0:1], mv[:, 0:1])
                nc.vector.scalar_tensor_tensor(
                    out=cst[:, 1:2], in0=mv[:, 0:1], scalar=mv[:, 0:1],
                    in1=mv[:, 1:2], op0=OP.mult, op1=OP.add,
                )
                pgs = psum_tiny.tile([8, 2], f32, tag="pgs")
                nc.tensor.matmul(pgs[:], cpack_t[:, 8:16], cst[:], start=True, stop=True)
                gr = gnp.tile([8, 2], f32, tag="gr")
                nc.vector.tensor_scalar_mul(gr[:, 0:2], pgs[:, 0:2], 1.0 / GSIZE)
                musq = gnp.tile([8, 1], f32, tag="musq")
                nc.vector.tensor_mul(musq[:], gr[:, 0:1], gr[:, 0:1])
                var = gnp.tile([8, 1], f32, tag="var")
                nc.vector.tensor_sub(var[:], gr[:, 1:2], musq[:])
                std = gnp.tile([8, 1], f32, tag="std")
                nc.scalar.activation(
                    out=std[:], in_=var[:], func=AF.Sqrt, bias=eps_t[:]
                )
                nc.vector.reciprocal(gr[:, 1:2], std[:])
                pbc = psum_tiny.tile([P, 2], f32, tag="pbc")
                nc.tensor.matmul(pbc[:], gmapT_t[:], gr[:], start=True, stop=True)
                scale_c = gnp.tile([P, 1], f32, tag="scale_c")
                nc.vector.tensor_mul(scale_c[:], pbc[:, 1:2], cpack_t[:, t:t + 1])
                mss = gnp.tile([P, 1], f32, tag="mss")
                nc.vector.tensor_mul(mss[:], pbc[:, 0:1], scale_c[:])
                bias_c = gnp.tile([P, 1], f32, tag="bias_c")
                nc.vector.tensor_sub(bias_c[:], cpack_t[:, CT + t:CT + t + 1], mss[:])
                nc.vector.tensor_scalar(
                    out=xn8_tiles[t // 2][:, t % 2, :], in0=xt[:],
                    scalar1=scale_c[:], scalar2=bias_c[:],
                    op0=OP.mult, op1=OP.add,
                )
                pe_warm(5)
            # switch the ACT table set to exp now (overlaps the qkv phase).
            # Input depends on the last GN sqrt so the scheduler cannot hoist
            # this above the sqrts (which would thrash table loads).
            nc.scalar.activation(out=warm_t[:], in_=std[0:1, :], func=AF.Exp)

        with tc.tile_pool(name="psum_av", bufs=2, space="PSUM") as psum_av:

            # ---- qkv helpers (fp8 DoubleRow) ----
            def emit_vt_tile(i):
                """vT tile [128, 8, 65]: per head h cols 0:64 = v chans, col 64
                = ones (softmax denominator via the AV matmul)."""
                ps = psum_big.tile([P, N], f32, tag="big", name=f"vtps{i}")
                pv = ps[:, 0:NCHUNK]
                for s in range(2):
                    nc.tensor.matmul(
                        pv,
                        xn8_tiles[s][:, :, i * P:(i + 1) * P],
                        w8_tiles[s][:, :, 2 * C:3 * C],
                        start=(s == 0), stop=False,
                        perf_mode=PM.DoubleRow,
                    )
                nc.tensor.matmul(pv, ones1_t[:], vb_t[:], start=False, stop=True)
                vt = vtp.tile([P, NHEADS, HD + 1], bf16, tag="vt", name=f"vt{i}")
                nc.scalar.activation(
                    out=vt[:, :, 0:HD], in_=pv.rearrange("p (h d) -> p h d", h=NHEADS),
                    func=AF.Identity,
                )
                nc.vector.tensor_copy(vt[:, :, HD:HD + 1], ones8_t[:, :, None])
                return vt

            # qk generation: per pair, q tile then k tile (fp8 DR, evac on ACT)
            qk_state = {}

            def qk_begin(p):
                qk_state[p] = {"chunk": 0, "ps": None, "sb": []}

            def qk_chunk(p):
                """Emit one DR matmul (of 8) for pair p; 4 per psum (s x j),
                q fully first then k. After each psum completes, ACT evacuates
                with the per-partition bias."""
                st = qk_state[p]
                c = st["chunk"]
                if c >= 8:
                    return
                st["chunk"] = c + 1
                which, cc = c // 4, c % 4
                off = which * C + p * P
                if cc == 0:
                    st["ps"] = psum_big.tile(
                        [P, N], f32, tag="big", name=f"qkps{p}_{which}"
                    )
                ps = st["ps"]
                s, j = cc // 2, cc % 2
                nc.tensor.matmul(
                    ps[:, j * NCHUNK:(j + 1) * NCHUNK],
                    w8_tiles[s][:, :, off:off + P],
                    xn8_tiles[s][:, :, j * NCHUNK:(j + 1) * NCHUNK],
                    start=(s == 0), stop=(s == 1),
                    perf_mode=PM.DoubleRow,
                )
                if cc == 3:
                    sb = qkp.tile([P, N], bf16, tag="qk", name=f"qk{p}_{which}")
                    # DVE (not ACT): head-A exps are latency-critical on ACT;
                    # this evac has half-a-pair of slack on the DVE queue
                    nc.vector.tensor_scalar_add(
                        sb[:], ps[:], cpack_t[:, 16 + which * 4 + p:17 + which * 4 + p]
                    )
                    st["sb"].append(sb)

            def qk_force(p):
                while qk_state[p]["chunk"] < 8:
                    qk_chunk(p)

            # ---- scores + exp (row-tiled PE pairs; ACT/DVE split) ----
            # DVE handles head B's exp via the Schraudolph bit trick, except
            # the m-steps listed in ACT_B (rebalance knob).
            ACT_B = (2, 5)
            exps = {}
            emitted = 0
            steps = [(p, i) for p in range(PAIRS) for i in range(MT)]

            def emit_scores_exp(p, i, q_t, k_t):
                psA = psum_big.tile([P, N], f32, tag="big", name=f"sA{p}_{i}")
                psB = psum_big.tile([P, N], f32, tag="big", name=f"sB{p}_{i}")
                for j in range(2):
                    nc.tensor.matmul(
                        psA[:, j * NCHUNK:(j + 1) * NCHUNK],
                        k_t[0:HD, i * P:(i + 1) * P],
                        q_t[0:HD, j * NCHUNK:(j + 1) * NCHUNK],
                        start=True, stop=True, tile_position=(0, 0),
                    )
                    nc.tensor.matmul(
                        psB[:, j * NCHUNK:(j + 1) * NCHUNK],
                        k_t[HD:P, i * P:(i + 1) * P],
                        q_t[HD:P, j * NCHUNK:(j + 1) * NCHUNK],
                        start=True, stop=True, tile_position=(64, 0),
                    )
                eA = expp.tile([P, N], bf16, tag="exp", name=f"eA{p}_{i}")
                nc.scalar.activation(out=eA[:], in_=psA[:], func=AF.Exp, scale=1.0 / 8.0)
                eB = expp.tile([P, N], bf16, tag="exp", name=f"eB{p}_{i}")
                if i in ACT_B:
                    nc.scalar.activation(out=eB[:], in_=psB[:], func=AF.Exp, scale=1.0 / 8.0)
                else:
                    nc.vector.tensor_scalar(
                        out=eB[:].bitcast(i16), in0=psB[:],
                        scalar1=SCH_S, scalar2=SCH_B, op0=OP.mult, op1=OP.add,
                    )
                return eA, eB

            def ensure_scores(n):
                nonlocal emitted
                while emitted < min(n, len(steps)):
                    p2, i2 = steps[emitted]
                    qk_force(p2)
                    exps[steps[emitted]] = emit_scores_exp(
                        p2, i2, *qk_state[p2]["sb"]
                    )
                    emitted += 1

            def emit_av(avt, p, i, h, start, stop):
                pair = exps[(p, i)]
                e = pair[h]
                if h == 1:
                    del exps[(p, i)]
                for j in range(2):
                    nc.tensor.matmul(
                        avt[:, j * NCHUNK:(j + 1) * NCHUNK],
                        vt_tiles[i][:, 2 * p + h, :],
                        e[:, j * NCHUNK:(j + 1) * NCHUNK],
                        start=start, stop=stop,
                    )

            def emit_norm(avt, p, h):
                """att8_mega[h'//4][(h'%2)*64 partitions, (h'//2)%2, :] =
                avt[0:64] * recip(avt[64]) for global head h' = 2p+h."""
                hh = 2 * p + h
                dinv = dvp.tile([1, N], f32, tag="dinv", name=f"dinv{hh}")
                nc.scalar.activation(out=dinv[:], in_=avt[HD:HD + 1, :], func=AF.Identity)
                nc.vector.reciprocal_approx_fast(dinv[:], dinv[:])
                dinvb = dvp.tile([HD, N], f32, tag="dinvb", name=f"dinvb{hh}")
                nc.gpsimd.partition_broadcast(dinvb[:], dinv[:])
                lo = (hh % 2) * HD
                nc.vector.tensor_mul(
                    att8_tiles[hh // 4][lo:lo + HD, (hh // 2) % 2, :],
                    avt[0:HD, :], dinvb[:],
                )

            att8_tiles = [
                att8p.tile([P, 2, N], fp8, tag="att8", name=f"att8_{m}")
                for m in range(2)
            ]

            # ---- flat software-pipelined stream ----
            LA = 1
            vt_tiles = [None] * MT
            proj_ps = {}
            qk_begin(0)
            qk_force(0)
            ensure_scores(LA)
            for i in range(MT):
                vt_tiles[i] = emit_vt_tile(i)
                if i in (2, 4, 6):
                    ensure_scores(LA + 1 + i // 2)

            def emit_proj(o, s, start, stop):
                if s == 0:
                    proj_ps[o] = psum_big.tile([P, N], f32, tag="big", name=f"pps{o}")
                pp = proj_ps[o]
                for j in range(2):
                    nc.tensor.matmul(
                        pp[:, j * NCHUNK:(j + 1) * NCHUNK],
                        wp8_tiles[s][:, :, o * P:(o + 1) * P],
                        att8_tiles[s][:, :, j * NCHUNK:(j + 1) * NCHUNK],
                        start=start, stop=stop,
                        perf_mode=PM.DoubleRow,
                    )

            for p in range(PAIRS):
                # head A AV trails the exp stream
                avt = psum_av.tile([HD + 1, N], f32, tag="av", name=f"avA{p}")
                for i in range(MT):
                    # last 2 steps: hold back exp lookahead so norm-A's ACT/DVE
                    # ops aren't queued behind lookahead exps (they gate the
                    # next pair's AV via the psum_av buffer rotation)
                    la = LA if i < MT - 2 else 0
                    ensure_scores(p * MT + i + 1 + la)
                    if p + 1 < PAIRS:
                        if i == 0:
                            qk_begin(p + 1)
                        qk_chunk(p + 1)
                    emit_av(avt, p, i, 0, start=(i == 0), stop=(i == MT - 1))
                emit_norm(avt, p, 0)
                ensure_scores(p * MT + MT + LA)
                if p == PAIRS - 1:
                    # all scores are emitted by now; big-pool slots are free.
                    # att8 mega 0 (heads 0-3) has been ready since pair 1 --
                    # pre-accumulate proj s=0 for 2 o-tiles as PE filler while
                    # the tail normalize chains run.
                    for o in range(2):
                        emit_proj(o, 0, start=True, stop=False)
                # head B AV blasts through retained exp tiles
                avt = psum_av.tile([HD + 1, N], f32, tag="av", name=f"avB{p}")
                for i in range(MT):
                    emit_av(avt, p, i, 1, start=(i == 0), stop=(i == MT - 1))
                    if i % 3 == 2:
                        ensure_scores(p * MT + MT + i // 3 + 1 + LA)
                emit_norm(avt, p, 1)

            # ---- proj s=1 + bias + residual + out DMA ----
            for o in range(CT):
                if o not in proj_ps:
                    emit_proj(o, 0, start=True, stop=False)
                emit_proj(o, 1, start=False, stop=True)
                ot = outp.tile([P, N], f32, tag="ot")
                nc.vector.scalar_tensor_tensor(
                    out=ot[:], in0=proj_ps[o][:], scalar=cpack_t[:, 24 + o:25 + o],
                    in1=xf_tiles[o][:], op0=OP.add, op1=OP.add,
                )
                nc.sync.dma_start(out_d[o * P:(o + 1) * P, :], ot[:])

    nc.compile()
    return nc


_CACHE = {}


def _get_program():
    if "nc" not in _CACHE:
        _CACHE["nc"] = build_program()
    return _CACHE["nc"]


def make_in_maps(x, gn_w, gn_b, qkv_w, qkv_b, proj_w, proj_b):
    B = x.shape[0]
    f = np.float32
    f8 = ml_dtypes.float8_e4m3
    # DoubleRow packing: contraction channel c -> (s=c//256, p=c%128, sub=(c//128)%2)
    wqkvT = np.ascontiguousarray(np.asarray(qkv_w, f).T)  # [512, 1536]
    w8 = np.ascontiguousarray(
        wqkvT.reshape(2, 2, P, 3 * C).transpose(0, 2, 1, 3)
    ).astype(f8)  # [2, 128, 2, 1536]
    wpT = np.ascontiguousarray(np.asarray(proj_w, f).T)  # [512, 512]
    wp8 = np.ascontiguousarray(
        wpT.reshape(2, 2, P, C).transpose(0, 2, 1, 3)
    ).astype(f8)  # [2, 128, 2, 512]
    qkb = np.asarray(qkv_b[:2 * C], f).reshape(8, P).T
    vb = np.asarray(qkv_b[2 * C:], f).reshape(1, C).astype(ml_dtypes.bfloat16)
    pb = np.asarray(proj_b, f).reshape(CT, P).T
    gnw = np.asarray(gn_w, f).reshape(CT, P).T
    gnb = np.asarray(gn_b, f).reshape(CT, P).T
    gmap = np.zeros((P, 8), f)
    gmap[np.arange(P), np.arange(P) // GSIZE] = 1.0
    gmapT = np.ascontiguousarray(gmap.T)
    cpack = np.ascontiguousarray(
        np.concatenate([gnw, gnb, gmap, qkb, pb], axis=1)
    )  # [128, 28]
    shared = dict(w8=w8, wp8=wp8, cpack=cpack, gmapT=gmapT, vb=vb)
    xs = np.asarray(x, f).reshape(B, C, N)
    return [
        dict(
            shared,
            xb=np.ascontiguousarray(xs[i]).astype(ml_dtypes.bfloat16),
            xf=np.ascontiguousarray(xs[i]),
        )
        for i in range(B)
    ]


def run(in_maps, trace=False, **kw):
    nc = _get_program()
    return run_bass_kernel_spmd(nc, in_maps, core_ids=list(range(len(in_maps))), trace=trace, **kw)


def kernel(x, gn_w, gn_b, qkv_w, qkv_b, proj_w, proj_b):
    x = np.asarray(x)
    B, c, h, w = x.shape
    in_maps = make_in_maps(x, gn_w, gn_b, qkv_w, qkv_b, proj_w, proj_b)
    res = run(in_maps)
    out = np.stack([res.results[i]["out"].reshape(c, h, w) for i in range(B)])
    return out.astype(np.float32)
